# revision 1
# baseline (speedup 1.0000x reference)
"""Trainium2 Bass kernel: depthwise 3x3 stencil conv (SAME, zero-pad) + residual.

Math (per image, per channel):
    out[h,w] = sum_{dh,dw} k[dh,dw] * x[h+dh-1, w+dw-1]  +  x[h,w]

The fixed stencil k = [[1,0,-1],[0,1,0],[-1,0,1]] is rank-2:
    k = outer((1,0,-1),(1,0,-1)) + center(1)
so with t[h,w] = x[h-1,w] - x[h+1,w] (vertical pass):
    out[h,w] = 2*x[h,w] + t[h,w-1] - t[h,w+1]

Mapping on one NeuronCore (batch is sharded 4 images/core across 8 cores):
  - layout: partitions = h (112 rows), free dim = (w,c) flattened (10752 f32)
    with 96-float zero pads on both ends (one w column, padded host-side)
  - vertical pass: banded 112x112 matmul on TensorE (PSUM, N=512 chunks)
  - PSUM -> SBUF t-slab copies on ScalarE
  - horizontal pass: two fused in-place DVE ops per piece:
        v   = 2*x + t@(w-1)      (scalar_tensor_tensor)
        out = v - t@(w+1)        (tensor_tensor)
  - straight contiguous HBM DMAs in/out (HWDGE)

fp32 self-loading matmuls can carry only ~1 semaphore wait (single EVENTS
slot in the LDWEIGHTS ISA struct), so DMA-completion waits are absorbed by
tiny dummy matmuls that read one column of the freshly loaded tile.
"""

import sys
import numpy as np

for _p in ("/opt/trn_rl_repo",):
    if _p not in sys.path:
        sys.path.insert(0, _p)

# ---------------- problem constants (hardcoded per contract) ----------------
N_CORES = 8
N, H, W, CH = 32, 112, 112, 96
IMGS_PER_CORE = N // N_CORES          # 4
ROWS = IMGS_PER_CORE * H              # 448 rows per core shard
FS = W * CH                           # 10752 floats per row
PAD = CH                              # one w column of zero padding
SLAB = FS + 2 * PAD                   # 10944
MM_N = 512                            # one PSUM bank of fp32
N_PIECES = 3                          # DVE piece split of the interior
PIECE = FS // N_PIECES                # 3584

_CACHE = {}
LAST_RESULTS = None  # BassKernelResults of the most recent run (for test.py)


def _build_bass(beta):
    """Raw-bass program with a hand-rolled static schedule.

    The walrus codegen used on this toolchain supports at most ONE semaphore
    wait per instruction, which rules out Tile's auto-generated multi-wait
    instructions.  Raw bass emits each wait as its own standalone wait_ge
    instruction on the consuming engine, which is always legal.

    Work is split into 8 units (4 images x 2 w-halves) with 4-deep slab
    buffering so load / matmul / copy / vector / store stages of different
    units overlap.  Per unit u:
        SP :  D(u)  x rows, w-halo cols -> xs[u%4]   (HBM -> SBUF, 2.4 MB)
        PE :  mm(u,g) ps[bank] = V^T @ xs[:, g]      (vertical pass, 11 groups)
        ACT:  cp(u,g) ts[u%4][:, g] <- ps[bank]      (PSUM -> SBUF)
        DVE:  op1  xs[96:5472] = beta*xs + ts[0:5376]        (v = 2x + t@w-1)
              op2  ts[96:5472] = xs[96:5472] - ts[192:5568]  (out = v - t@w+1)
              drain -> inc dve sem
        SP :  O(u)  ts[96:5472] -> out rows/cols     (SBUF -> HBM)
    """
    from concourse import bass, mybir

    f32 = mybir.dt.float32
    nc = bass.Bass(debug=False)
    x_d = nc.declare_dram_parameter("x", [ROWS, SLAB], f32, isOutput=False)
    v_d = nc.declare_dram_parameter("vmat", [H, H], f32, isOutput=False)
    out_d = nc.declare_dram_parameter("out", [ROWS, FS], f32, isOutput=True)

    WHALF = W // 2            # 56 output columns per unit
    USLAB = (WHALF + 2) * CH  # 5568 slab floats (1 w-col halo each side)
    UINT = WHALF * CH         # 5376 interior floats
    NU = IMGS_PER_CORE * 2    # 8 units
    NS = 4                    # slab sets in flight

    groups = []
    off = 0
    while off < USLAB:
        n = min(MM_N, USLAB - off)
        groups.append((off, n))
        off += n
    n_g = len(groups)  # 11

    vt = nc.alloc_sbuf_tensor("vt", [H, H], f32)
    xs = [nc.alloc_sbuf_tensor(f"xs{k}", [H, USLAB], f32) for k in range(NS)]
    ts = [nc.alloc_sbuf_tensor(f"ts{k}", [H, USLAB], f32) for k in range(NS)]
    NB = 8
    ps = [nc.alloc_psum_tensor(f"ps{b}", [H, MM_N], f32) for b in range(NB)]

    def unit_rows(u):
        i = u // 2
        return i * H, (i + 1) * H

    def unit_slab_col(u):
        # start column of the unit's slab inside the padded x row [ROWS, SLAB]
        return (u % 2) * WHALF * CH  # 0 or 5376

    from contextlib import ExitStack

    with (
        nc.Block(no_gpsimd_drain=True) as block,
        nc.semaphore("s_vt") as s_vt,
        nc.semaphore("s_pe") as s_pe,
        nc.semaphore("s_act") as s_act,
        nc.semaphore("s_dve") as s_dve,
        ExitStack() as _sems,
    ):
        # Per-slab-set DMA completion semaphores.  A single cumulative DMA
        # semaphore would race: concurrent DMAs can complete out of issue
        # order, so "sem >= 16*(u+1)" could be satisfied by a LATER unit's
        # transfer while unit u's data is still in flight.  Per-set sems are
        # safe because successive users of one set never overlap in flight.
        s_din = [_sems.enter_context(nc.semaphore(f"s_din{k}")) for k in range(NS)]
        s_din2 = [_sems.enter_context(nc.semaphore(f"s_dinb{k}")) for k in range(NS)]
        s_dout = [_sems.enter_context(nc.semaphore(f"s_dout{k}")) for k in range(NS)]

        @block.sync
        def _(sp: bass.BassEngine):
            sp.dma_start(out=vt[:, :], in_=v_d[:, :]).then_inc(s_vt, 16)

            # loads are split in two halves on separate sems so the PE can
            # start on the first half; LSPLIT is a matmul-group boundary
            LSPLIT = 5 * MM_N  # 2560

            def load(u):
                r0, r1 = unit_rows(u)
                c0 = unit_slab_col(u)
                sp.dma_start(
                    out=xs[u % NS][:, 0:LSPLIT], in_=x_d[r0:r1, c0 : c0 + LSPLIT]
                ).then_inc(s_din[u % NS], 16)
                sp.dma_start(
                    out=xs[u % NS][:, LSPLIT:USLAB],
                    in_=x_d[r0:r1, c0 + LSPLIT : c0 + USLAB],
                ).then_inc(s_din2[u % NS], 16)

            for u in range(min(NS, NU)):
                load(u)
            for u in range(NU):
                r0, r1 = unit_rows(u)
                oc0 = (u % 2) * UINT
                # store unit u once its DVE drain fired
                sp.wait_ge(s_dve, u + 1)
                sp.dma_start(
                    out=out_d[r0:r1, oc0 : oc0 + UINT],
                    in_=ts[u % NS][:, PAD : PAD + UINT],
                ).then_inc(s_dout[u % NS], 16)
                nxt = u + NS
                if nxt < NU:
                    # reload xs[u%NS]: PE reads of unit u must be done (DVE
                    # covered by the store wait above)
                    sp.wait_ge(s_pe, n_g * (u + 1))
                    load(nxt)
            for k in range(NS):
                sp.wait_ge(s_dout[k], 16 * (NU // NS))

        @block.tensor
        def _(pe: bass.BassEngine):
            pe.wait_ge(s_vt, 16)
            for u in range(NU):
                pe.wait_ge(s_din[u % NS], 16 * (u // NS + 1))
                for g, (goff, gn) in enumerate(groups):
                    if g == 5:  # groups 5.. read past LSPLIT
                        pe.wait_ge(s_din2[u % NS], 16 * (u // NS + 1))
                    idx = u * n_g + g
                    if idx >= NB:
                        # psum bank reuse: the copy that read it must be done
                        pe.wait_ge(s_act, idx - NB + 1)
                    pe.matmul(
                        out=ps[idx % NB][0:H, 0:gn],
                        lhsT=vt[:, :],
                        rhs=xs[u % NS][:, goff : goff + gn],
                        start=True,
                        stop=True,
                    ).then_inc(s_pe, 1)

        @block.scalar
        def _(act: bass.BassEngine):
            for u in range(NU):
                if u >= NS:
                    # ts slab reuse: unit u-NS's DVE write and store DMA done
                    act.wait_ge(s_dve, u - NS + 1)
                    act.wait_ge(s_dout[u % NS], 16 * (u // NS))
                for g, (goff, gn) in enumerate(groups):
                    idx = u * n_g + g
                    act.wait_ge(s_pe, idx + 1)
                    act.copy(
                        out=ts[u % NS][:, goff : goff + gn],
                        in_=ps[idx % NB][0:H, 0:gn],
                    ).then_inc(s_act, 1)

        @block.vector
        def _(dve: bass.BassEngine):
            for u in range(NU):
                # all matmul groups of unit u must have read xs before op1
                # overwrites it, and all copies must have produced ts
                dve.wait_ge(s_pe, n_g * (u + 1))
                dve.wait_ge(s_act, n_g * (u + 1))
                dve.scalar_tensor_tensor(
                    out=xs[u % NS][:, PAD : PAD + UINT],
                    in0=xs[u % NS][:, PAD : PAD + UINT],
                    scalar=float(beta),
                    in1=ts[u % NS][:, 0:UINT],
                    op0=mybir.AluOpType.mult,
                    op1=mybir.AluOpType.add,
                )
                dve.tensor_tensor(
                    out=ts[u % NS][:, PAD : PAD + UINT],
                    in0=xs[u % NS][:, PAD : PAD + UINT],
                    in1=ts[u % NS][:, 2 * PAD : 2 * PAD + UINT],
                    op=mybir.AluOpType.subtract,
                )
                dve.drain().then_inc(s_dve, 1)

    return nc


def _stencil_params(kern):
    """Validate the depthwise kernel and extract (vertical profile a, beta).

    Requires: channels identical, k[:,2] == -k[:,0], k[0,1] == k[2,1] == 0.
    Returns (a, beta) with a = k[:,0] (vertical mixing profile) and
    beta = k[1,1] + 1 (center coefficient incl. the residual).
    """
    k = np.asarray(kern, dtype=np.float32)
    if k.ndim != 4 or k.shape != (3, 3, 1, CH):
        return None
    if not np.all(k == k[:, :, :, :1]):
        return None
    k2 = k[:, :, 0, 0]
    if not (np.all(k2[:, 2] == -k2[:, 0]) and k2[0, 1] == 0 and k2[2, 1] == 0):
        return None
    return k2[:, 0].copy(), float(k2[1, 1]) + 1.0


def _numpy_fallback(x, kern):
    """Straightforward shifted-add implementation (safety net only)."""
    k = np.asarray(kern, dtype=np.float32)[:, :, 0, :]  # (3,3,CH)
    xp = np.pad(x, ((0, 0), (1, 1), (1, 1), (0, 0)))
    out = x.astype(np.float32).copy()
    for dh in range(3):
        for dw in range(3):
            out += k[dh, dw] * xp[:, dh : dh + H, dw : dw + W, :]
    return out


def _ensure_ntff_hook():
    """The agent image's antenv lacks axon_hooks; synthesize it so
    run_bass_kernel_spmd(trace=True) can reach the NTFF profiler."""
    import types

    if "antenv.axon_hooks" in sys.modules:
        return
    import antenv

    mod = types.ModuleType("antenv.axon_hooks")
    state = {}
    mod.set_axon_ntff_profile_hook = lambda h: state.__setitem__("h", h)
    mod.get_axon_ntff_profile_hook = lambda: state.get("h")
    sys.modules["antenv.axon_hooks"] = mod
    antenv.axon_hooks = mod
    try:
        if "/root/.axon_site" not in sys.path:
            sys.path.insert(0, "/root/.axon_site")
        from trn_agent_boot.trn_boot import _ntff_profile_via_ctypes

        hook = _ntff_profile_via_ctypes("/opt/axon/libaxon_pjrt.so")
        if hook is not None:
            mod.set_axon_ntff_profile_hook(hook)
    except Exception:
        pass


def _run_on_hw(x, a, beta, trace=False):
    global LAST_RESULTS
    if trace:
        _ensure_ntff_hook()
    from concourse.bass_utils import run_bass_kernel_spmd

    # vertical banded matrix: V[i, j] = coeff of x-row i in t-row j
    V = np.zeros((H, H), dtype=np.float32)
    idx = np.arange(H)
    V[idx[:-1] + 1, idx[:-1]] += a[2]   # i = j+1
    V[idx, idx] += a[1]                 # i = j
    V[idx[1:] - 1, idx[1:]] += a[0]     # i = j-1

    key = (a.tobytes(), float(beta))
    if key not in _CACHE:
        _CACHE[key] = _build_bass(beta)
    nc = _CACHE[key]

    # host-side zero padding of one w column on each side (pads the slab so
    # the device needs no memsets)
    xp = np.zeros((N_CORES, ROWS, SLAB), dtype=np.float32)
    xp[:, :, PAD : PAD + FS] = x.reshape(N_CORES, ROWS, FS)
    in_maps = [{"x": xp[c], "vmat": V} for c in range(N_CORES)]
    res = run_bass_kernel_spmd(nc, in_maps, list(range(N_CORES)), trace=trace)
    LAST_RESULTS = res
    out = np.stack([res.results[c]["out"] for c in range(N_CORES)])
    return out.reshape(N, H, W, CH)


def kernel(x, kernel=None, _trace=False, **_unused):
    x = np.ascontiguousarray(np.asarray(x, dtype=np.float32))
    assert x.shape == (N, H, W, CH), f"unexpected x shape {x.shape}"
    if kernel is None:
        base = np.array(
            [[1.0, 0.0, -1.0], [0.0, 1.0, 0.0], [-1.0, 0.0, 1.0]], dtype=np.float32
        )
        kernel = np.tile(base[:, :, None, None], (1, 1, 1, CH))
    params = _stencil_params(kernel)
    if params is None:
        return _numpy_fallback(x, kernel)
    a, beta = params
    return _run_on_hw(x, a, beta, trace=_trace)


if __name__ == "__main__":
    xs = np.random.randn(N, H, W, CH).astype(np.float32)
    out = kernel(xs)
    print(out.shape, out.dtype)



# revision 3
# speedup vs baseline: 1.6102x; 1.6102x over previous
"""Trainium2 Bass kernel: depthwise 3x3 stencil conv (SAME, zero-pad) + residual.

Math (per image, per channel):
    out[h,w] = sum_{dh,dw} k[dh,dw] * x[h+dh-1, w+dw-1]  +  x[h,w]

The fixed stencil k = [[1,0,-1],[0,1,0],[-1,0,1]] is rank-2:
    k = outer((1,0,-1),(1,0,-1)) + center(1)
so with t[h,w] = x[h-1,w] - x[h+1,w] (vertical pass):
    out[h,w] = 2*x[h,w] + t[h,w-1] - t[h,w+1]

This version runs the whole device pipeline in bf16, which halves HBM
traffic (the kernel is memory-bound) and doubles DVE throughput:

  - host converts x to bf16 and pads one w column of zeros on each side
  - device computes out/2 = (beta/2)*x + 0.5*t[w-1] - 0.5*t[w+1]:
      * vertical pass t = V^T @ x on TensorE (bf16 matmul, f32 PSUM)
      * ACT copies PSUM -> SBUF with scale=0.5 and bf16 downcast, in
        2048-column multi-bank reads (amortizes per-instruction overhead)
      * for beta == 2 both horizontal ops are then plain tensor_tensor
        adds/subs on DVE, whose all-2-byte operands hit the 2x_1P
        double-rate mode
  - host upconverts and multiplies by 2 (exact: power of two in fp32)

Layout on one NeuronCore (batch sharded 4 images/core across 8 cores):
partitions = h (112 rows), free dim = (w,c) flattened + 96-elem zero pad
per side (slab = 10944 bf16). Each image has its own xs/ts slab pair in
SBUF (4 sets, ~175 KB/partition total) so there is no buffer reuse and
the whole input streams in up front.
"""

import sys
import numpy as np

for _p in ("/opt/trn_rl_repo",):
    if _p not in sys.path:
        sys.path.insert(0, _p)

# ---------------- problem constants (hardcoded per contract) ----------------
N_CORES = 8
N, H, W, CH = 32, 112, 112, 96
IMGS_PER_CORE = N // N_CORES          # 4
ROWS = IMGS_PER_CORE * H              # 448 rows per core shard
FS = W * CH                           # 10752 elems per row
PAD = CH                              # one w column of zero padding
SLAB = FS + 2 * PAD                   # 10944
MM_N = 512                            # one PSUM bank of fp32
CHUNK = 2048                          # ACT copy width (4 PSUM banks)

_CACHE = {}
LAST_RESULTS = None  # BassKernelResults of the most recent run (for test.py)


def _build_bass(beta):
    """Raw-bass program with a hand-rolled static schedule, all bf16.

    Per image u (4 per core, each with its own xs/ts slab set):
        SP :  3 load DMAs (HBM -> SBUF), piece cols [0,4096/8192/10944)
        PE :  22 matmuls ps = V^T @ xs[:, g*512:...] (f32 PSUM, 512 cols)
        ACT:  6 copies ts[:, chunk] = 0.5 * ps (multi-bank read, bf16 out)
        DVE:  op1  xs[96:10848] += ts[0:10752]          (v = x + t@w-1)
              op2  ts[96:10848] = xs[96:...] - ts[192:] (out/2 = v - t@w+1)
              drain -> inc dve sem
        SP :  store ts[96:10848] -> out rows (SBUF -> HBM)

    PSUM is split into two 4-bank halves psA/psB; chunk c = 6u+j uses
    ps[c%2], so matmuls of chunk c wait for the copy of chunk c-2.
    """
    from concourse import bass, mybir

    bf16 = mybir.dt.bfloat16
    f32 = mybir.dt.float32
    nc = bass.Bass(debug=False)
    x_d = nc.declare_dram_parameter("x", [ROWS, SLAB], bf16, isOutput=False)
    v_d = nc.declare_dram_parameter("vmat", [H, H], bf16, isOutput=False)
    out_d = nc.declare_dram_parameter("out", [ROWS, FS], bf16, isOutput=True)

    NU = IMGS_PER_CORE            # 4 units = 4 images
    n_g = (SLAB + MM_N - 1) // MM_N       # 22 matmul groups (21x512 + 192)
    n_ch = (SLAB + CHUNK - 1) // CHUNK    # 6 copy chunks (5x2048 + 704)
    # load piece boundaries (cols): chunk-aligned so PE gating is simple
    PIECES = [(0, 4096), (4096, 8192), (8192, SLAB)]

    vt = nc.alloc_sbuf_tensor("vt", [H, H], bf16)
    xs = [nc.alloc_sbuf_tensor(f"xs{k}", [H, SLAB], bf16) for k in range(NU)]
    ts = [nc.alloc_sbuf_tensor(f"ts{k}", [H, SLAB], bf16) for k in range(NU)]
    ps = [nc.alloc_psum_tensor(f"ps{b}", [H, CHUNK], f32) for b in range(2)]

    from contextlib import ExitStack

    with (
        nc.Block(no_gpsimd_drain=True) as block,
        nc.semaphore("s_vt") as s_vt,
        nc.semaphore("s_pe") as s_pe,
        nc.semaphore("s_act") as s_act,
        nc.semaphore("s_dve") as s_dve,
        ExitStack() as _sems,
    ):
        # per-piece DMA completion semaphores (completions of concurrent
        # DMAs can land out of issue order; per-piece sems are exact)
        s_din = [
            [_sems.enter_context(nc.semaphore(f"s_din{u}_{p}")) for p in range(3)]
            for u in range(NU)
        ]
        s_dout = [_sems.enter_context(nc.semaphore(f"s_dout{u}")) for u in range(NU)]

        @block.sync
        def _(sp: bass.BassEngine):
            sp.dma_start(out=vt[:, :], in_=v_d[:, :]).then_inc(s_vt, 16)
            for u in range(NU):
                r0 = u * H
                for p, (c0, c1) in enumerate(PIECES):
                    sp.dma_start(
                        out=xs[u][:, c0:c1], in_=x_d[r0 : r0 + H, c0:c1]
                    ).then_inc(s_din[u][p], 16)
            for u in range(NU):
                r0 = u * H
                sp.wait_ge(s_dve, u + 1)
                sp.dma_start(
                    out=out_d[r0 : r0 + H, 0:FS],
                    in_=ts[u][:, PAD : PAD + FS],
                ).then_inc(s_dout[u], 16)
            for u in range(NU):
                sp.wait_ge(s_dout[u], 16)

        @block.tensor
        def _(pe: bass.BassEngine):
            pe.wait_ge(s_vt, 16)
            for u in range(NU):
                for g in range(n_g):
                    goff = g * MM_N
                    gn = min(MM_N, SLAB - goff)
                    j = g // 4          # chunk within image
                    c = u * n_ch + j    # global chunk index
                    if g % 4 == 0:
                        # piece gating: chunks 0/2/4 start on a new piece
                        if j in (0, 2, 4):
                            pe.wait_ge(s_din[u][j // 2], 16)
                        # psum half reuse: chunk c-2's copy must be done
                        if c >= 2:
                            pe.wait_ge(s_act, c - 1)
                    pe.matmul(
                        out=ps[c % 2][0:H, (g % 4) * MM_N : (g % 4) * MM_N + gn],
                        lhsT=vt[:, :],
                        rhs=xs[u][:, goff : goff + gn],
                        start=True,
                        stop=True,
                    ).then_inc(s_pe, 1)

        @block.scalar
        def _(act: bass.BassEngine):
            for u in range(NU):
                for j in range(n_ch):
                    c0 = j * CHUNK
                    csz = min(CHUNK, SLAB - c0)
                    c = u * n_ch + j
                    # groups 4j..min(4j+3, n_g-1) of this image must be done
                    g_hi = min(4 * (j + 1), n_g)
                    act.wait_ge(s_pe, u * n_g + g_hi)
                    act.mul(
                        ts[u][:, c0 : c0 + csz], ps[c % 2][0:H, 0:csz], 0.5
                    ).then_inc(s_act, 1)

        @block.vector
        def _(dve: bass.BassEngine):
            for u in range(NU):
                # all matmuls of image u must have read xs before op1
                # overwrites it; all chunk copies must have produced ts
                dve.wait_ge(s_pe, n_g * (u + 1))
                dve.wait_ge(s_act, n_ch * (u + 1))
                if beta == 2.0:
                    dve.tensor_tensor(
                        out=xs[u][:, PAD : PAD + FS],
                        in0=xs[u][:, PAD : PAD + FS],
                        in1=ts[u][:, 0:FS],
                        op=mybir.AluOpType.add,
                    )
                else:
                    dve.scalar_tensor_tensor(
                        out=xs[u][:, PAD : PAD + FS],
                        in0=xs[u][:, PAD : PAD + FS],
                        scalar=float(beta) / 2.0,
                        in1=ts[u][:, 0:FS],
                        op0=mybir.AluOpType.mult,
                        op1=mybir.AluOpType.add,
                    )
                dve.tensor_tensor(
                    out=ts[u][:, PAD : PAD + FS],
                    in0=xs[u][:, PAD : PAD + FS],
                    in1=ts[u][:, 2 * PAD : 2 * PAD + FS],
                    op=mybir.AluOpType.subtract,
                )
                dve.drain().then_inc(s_dve, 1)

    return nc


def _stencil_params(kern):
    """Validate the depthwise kernel and extract (vertical profile a, beta).

    Requires: channels identical, k[:,2] == -k[:,0], k[0,1] == k[2,1] == 0.
    Returns (a, beta) with a = k[:,0] (vertical mixing profile) and
    beta = k[1,1] + 1 (center coefficient incl. the residual).
    """
    k = np.asarray(kern, dtype=np.float32)
    if k.ndim != 4 or k.shape != (3, 3, 1, CH):
        return None
    if not np.all(k == k[:, :, :, :1]):
        return None
    k2 = k[:, :, 0, 0]
    if not (np.all(k2[:, 2] == -k2[:, 0]) and k2[0, 1] == 0 and k2[2, 1] == 0):
        return None
    return k2[:, 0].copy(), float(k2[1, 1]) + 1.0


def _numpy_fallback(x, kern):
    """Straightforward shifted-add implementation (safety net only)."""
    k = np.asarray(kern, dtype=np.float32)[:, :, 0, :]  # (3,3,CH)
    xp = np.pad(x, ((0, 0), (1, 1), (1, 1), (0, 0)))
    out = x.astype(np.float32).copy()
    for dh in range(3):
        for dw in range(3):
            out += k[dh, dw] * xp[:, dh : dh + H, dw : dw + W, :]
    return out


def _ensure_ntff_hook():
    """The agent image's antenv lacks axon_hooks; synthesize it so
    run_bass_kernel_spmd(trace=True) can reach the NTFF profiler."""
    import types

    if "antenv.axon_hooks" in sys.modules:
        return
    import antenv

    mod = types.ModuleType("antenv.axon_hooks")
    state = {}
    mod.set_axon_ntff_profile_hook = lambda h: state.__setitem__("h", h)
    mod.get_axon_ntff_profile_hook = lambda: state.get("h")
    sys.modules["antenv.axon_hooks"] = mod
    antenv.axon_hooks = mod
    try:
        if "/root/.axon_site" not in sys.path:
            sys.path.insert(0, "/root/.axon_site")
        from trn_agent_boot.trn_boot import _ntff_profile_via_ctypes

        hook = _ntff_profile_via_ctypes("/opt/axon/libaxon_pjrt.so")
        if hook is not None:
            mod.set_axon_ntff_profile_hook(hook)
    except Exception:
        pass


def _run_on_hw(x, a, beta, trace=False):
    global LAST_RESULTS
    if trace:
        _ensure_ntff_hook()
    import ml_dtypes
    from concourse.bass_utils import run_bass_kernel_spmd

    bf16 = ml_dtypes.bfloat16

    # vertical banded matrix: V[i, j] = coeff of x-row i in t-row j
    V = np.zeros((H, H), dtype=np.float32)
    idx = np.arange(H)
    V[idx[:-1] + 1, idx[:-1]] += a[2]   # i = j+1
    V[idx, idx] += a[1]                 # i = j
    V[idx[1:] - 1, idx[1:]] += a[0]     # i = j-1
    Vb = V.astype(bf16)

    key = (a.tobytes(), float(beta))
    if key not in _CACHE:
        _CACHE[key] = _build_bass(float(beta))
    nc = _CACHE[key]

    # host-side bf16 conversion + zero padding of one w column each side
    xp = np.zeros((N_CORES, ROWS, SLAB), dtype=bf16)
    xp[:, :, PAD : PAD + FS] = x.reshape(N_CORES, ROWS, FS).astype(bf16)
    in_maps = [{"x": xp[c], "vmat": Vb} for c in range(N_CORES)]
    res = run_bass_kernel_spmd(nc, in_maps, list(range(N_CORES)), trace=trace)
    LAST_RESULTS = res
    # device returned out/2 in bf16; x2 after upconvert is exact
    out = np.stack(
        [np.asarray(res.results[c]["out"], dtype=np.float32) for c in range(N_CORES)]
    )
    out *= 2.0
    return out.reshape(N, H, W, CH)


def kernel(x, kernel=None, _trace=False, **_unused):
    x = np.ascontiguousarray(np.asarray(x, dtype=np.float32))
    assert x.shape == (N, H, W, CH), f"unexpected x shape {x.shape}"
    if kernel is None:
        base = np.array(
            [[1.0, 0.0, -1.0], [0.0, 1.0, 0.0], [-1.0, 0.0, 1.0]], dtype=np.float32
        )
        kernel = np.tile(base[:, :, None, None], (1, 1, 1, CH))
    params = _stencil_params(kernel)
    if params is None:
        return _numpy_fallback(x, kernel)
    a, beta = params
    return _run_on_hw(x, a, beta, trace=_trace)


if __name__ == "__main__":
    xs = np.random.randn(N, H, W, CH).astype(np.float32)
    out = kernel(xs)
    print(out.shape, out.dtype)


# revision 4
# speedup vs baseline: 1.8234x; 1.1324x over previous
"""Trainium2 Bass kernel: depthwise 3x3 stencil conv (SAME, zero-pad) + residual.

Math (per image, per channel):
    out[h,w] = sum_{dh,dw} k[dh,dw] * x[h+dh-1, w+dw-1]  +  x[h,w]

The fixed stencil k = [[1,0,-1],[0,1,0],[-1,0,1]] is rank-2:
    k = outer((1,0,-1),(1,0,-1)) + center(1)
so with t[h,w] = x[h-1,w] - x[h+1,w] (vertical pass):
    out[h,w] = 2*x[h,w] + t[h,w-1] - t[h,w+1]

All-bf16 device pipeline (memory-bound problem: bf16 halves HBM traffic and
doubles DVE throughput via the 2x_1P packed mode):

  - host converts x to bf16 and pads one w column of zeros on each side
  - device computes out/2 = (beta/2)*x + 0.5*t[w-1] - 0.5*t[w+1]:
      * vertical pass t = V^T @ x on TensorE (bf16 matmul, f32 PSUM)
      * ACT copies PSUM -> SBUF with scale=0.5 and bf16 downcast, in
        2048-column multi-bank reads (amortizes per-instruction overhead)
      * for beta == 2 both horizontal ops are then plain tensor_tensor
        adds/subs on DVE whose all-2-byte operands run at 2 elem/cycle
  - host upconverts and multiplies by 2 (exact: power of two in fp32)

Work is split into 8 units (4 images x 2 w-halves) for pipeline
granularity; every unit has its own xs/ts slab pair in SBUF (16 slabs,
~178 KB/partition) so there is no buffer reuse and the whole input
streams in up front. PE semaphore increments are per-chunk (not per
matmul) so consecutive LDWEIGHTS/MATMUL pairs can pipeline in the PE's
reorder window.
"""

import sys
import numpy as np

for _p in ("/opt/trn_rl_repo",):
    if _p not in sys.path:
        sys.path.insert(0, _p)

# ---------------- problem constants (hardcoded per contract) ----------------
N_CORES = 8
N, H, W, CH = 32, 112, 112, 96
IMGS_PER_CORE = N // N_CORES          # 4
ROWS = IMGS_PER_CORE * H              # 448 rows per core shard
FS = W * CH                           # 10752 elems per row
PAD = CH                              # one w column of zero padding
SLAB = FS + 2 * PAD                   # 10944
MM_N = 512                            # one PSUM bank of fp32
CHUNK = 2048                          # ACT copy width (4 PSUM banks)

WHALF = W // 2                        # 56 output columns per unit
USLAB = (WHALF + 2) * CH              # 5568 slab cols (1 w-col halo each side)
UINT = WHALF * CH                     # 5376 interior cols
NU = IMGS_PER_CORE * 2                # 8 units
N_CH = (USLAB + CHUNK - 1) // CHUNK   # 3 chunks (2048, 2048, 1472)
LSPLIT = CHUNK                        # first load piece = first chunk

_CACHE = {}
LAST_RESULTS = None  # BassKernelResults of the most recent run (for test.py)


def _build_bass(beta):
    """Raw-bass program with a hand-rolled static schedule, all bf16.

    Per unit u (image u//2, w-half u%2):
        SP :  2 load DMAs (cols [0,2048) then [2048,5568))
        PE :  11 matmuls ps = V^T @ xs[:, g*512:...], sem inc per chunk
        ACT:  3 copies ts[:, chunk] = 0.5 * ps (multi-bank read, bf16 out)
        DVE:  op1  xs[96:5472] += ts[0:5376]           (v = x + t@w-1)
              op2  ts[96:5472] = xs[96:...] - ts[192:] (out/2 = v - t@w+1)
              drain -> inc dve sem
        SP :  store ts[96:5472] -> out rows (SBUF -> HBM)

    PSUM is split into two 4-bank halves psA/psB; chunk c = 3u+j uses
    ps[c%2], so matmuls of chunk c wait for the copy of chunk c-2.
    """
    from concourse import bass, mybir

    bf16 = mybir.dt.bfloat16
    f32 = mybir.dt.float32
    nc = bass.Bass(debug=False)
    x_d = nc.declare_dram_parameter("x", [ROWS, SLAB], bf16, isOutput=False)
    v_d = nc.declare_dram_parameter("vmat", [H, H], bf16, isOutput=False)
    out_d = nc.declare_dram_parameter("out", [ROWS, FS], bf16, isOutput=True)

    n_g = (USLAB + MM_N - 1) // MM_N      # 11 matmul groups (10x512 + 448)
    CHUNK_G = [(0, 4), (4, 8), (8, n_g)]  # group ranges per chunk

    vt = nc.alloc_sbuf_tensor("vt", [H, H], bf16)
    xs = [nc.alloc_sbuf_tensor(f"xs{k}", [H, USLAB], bf16) for k in range(NU)]
    ts = [nc.alloc_sbuf_tensor(f"ts{k}", [H, USLAB], bf16) for k in range(NU)]
    ps = [nc.alloc_psum_tensor(f"ps{b}", [H, CHUNK], f32) for b in range(2)]

    def unit_rows(u):
        i = u // 2
        return i * H, (i + 1) * H

    def unit_slab_col(u):
        # start column of the unit's slab inside the padded x row [ROWS, SLAB]
        return (u % 2) * WHALF * CH  # 0 or 5376

    from contextlib import ExitStack

    with (
        nc.Block(no_gpsimd_drain=True) as block,
        nc.semaphore("s_vt") as s_vt,
        nc.semaphore("s_pe") as s_pe,
        nc.semaphore("s_act") as s_act,
        nc.semaphore("s_dve") as s_dve,
        ExitStack() as _sems,
    ):
        # per-piece DMA completion semaphores (completions of concurrent
        # DMAs can land out of issue order; per-piece sems are exact)
        s_din = [
            [_sems.enter_context(nc.semaphore(f"s_din{u}_{p}")) for p in range(2)]
            for u in range(NU)
        ]
        s_dout = [_sems.enter_context(nc.semaphore(f"s_dout{u}")) for u in range(NU)]

        @block.sync
        def _(sp: bass.BassEngine):
            sp.dma_start(out=vt[:, :], in_=v_d[:, :]).then_inc(s_vt, 16)
            for u in range(NU):
                r0, r1 = unit_rows(u)
                c0 = unit_slab_col(u)
                sp.dma_start(
                    out=xs[u][:, 0:LSPLIT], in_=x_d[r0:r1, c0 : c0 + LSPLIT]
                ).then_inc(s_din[u][0], 16)
                sp.dma_start(
                    out=xs[u][:, LSPLIT:USLAB],
                    in_=x_d[r0:r1, c0 + LSPLIT : c0 + USLAB],
                ).then_inc(s_din[u][1], 16)
            for u in range(NU):
                r0, r1 = unit_rows(u)
                oc0 = (u % 2) * UINT
                sp.wait_ge(s_dve, u + 1)
                sp.dma_start(
                    out=out_d[r0:r1, oc0 : oc0 + UINT],
                    in_=ts[u][:, PAD : PAD + UINT],
                ).then_inc(s_dout[u], 16)
            for u in range(NU):
                sp.wait_ge(s_dout[u], 16)

        @block.tensor
        def _(pe: bass.BassEngine):
            pe.wait_ge(s_vt, 16)
            for u in range(NU):
                for j, (g_lo, g_hi) in enumerate(CHUNK_G):
                    c = u * N_CH + j  # global chunk index
                    pe.wait_ge(s_din[u][0 if j == 0 else 1], 16)
                    if c >= 2:
                        # psum half reuse: chunk c-2's copy must be done
                        pe.wait_ge(s_act, c - 1)
                    for g in range(g_lo, g_hi):
                        goff = g * MM_N
                        gn = min(MM_N, USLAB - goff)
                        mm = pe.matmul(
                            out=ps[c % 2][
                                0:H, (g - g_lo) * MM_N : (g - g_lo) * MM_N + gn
                            ],
                            lhsT=vt[:, :],
                            rhs=xs[u][:, goff : goff + gn],
                            start=True,
                            stop=True,
                        )
                        if g == g_hi - 1:
                            mm.then_inc(s_pe, 1)

        @block.scalar
        def _(act: bass.BassEngine):
            for u in range(NU):
                for j in range(N_CH):
                    c0 = j * CHUNK
                    csz = min(CHUNK, USLAB - c0)
                    c = u * N_CH + j
                    act.wait_ge(s_pe, c + 1)
                    act.mul(
                        ts[u][:, c0 : c0 + csz], ps[c % 2][0:H, 0:csz], 0.5
                    ).then_inc(s_act, 1)

        @block.vector
        def _(dve: bass.BassEngine):
            for u in range(NU):
                # all chunks of unit u must be matmul'd (xs fully read
                # before op1 overwrites it) and copied (ts ready)
                dve.wait_ge(s_pe, N_CH * (u + 1))
                dve.wait_ge(s_act, N_CH * (u + 1))
                if beta == 2.0:
                    dve.tensor_tensor(
                        out=xs[u][:, PAD : PAD + UINT],
                        in0=xs[u][:, PAD : PAD + UINT],
                        in1=ts[u][:, 0:UINT],
                        op=mybir.AluOpType.add,
                    )
                else:
                    dve.scalar_tensor_tensor(
                        out=xs[u][:, PAD : PAD + UINT],
                        in0=xs[u][:, PAD : PAD + UINT],
                        scalar=float(beta) / 2.0,
                        in1=ts[u][:, 0:UINT],
                        op0=mybir.AluOpType.mult,
                        op1=mybir.AluOpType.add,
                    )
                dve.tensor_tensor(
                    out=ts[u][:, PAD : PAD + UINT],
                    in0=xs[u][:, PAD : PAD + UINT],
                    in1=ts[u][:, 2 * PAD : 2 * PAD + UINT],
                    op=mybir.AluOpType.subtract,
                )
                dve.drain().then_inc(s_dve, 1)

    return nc


def _stencil_params(kern):
    """Validate the depthwise kernel and extract (vertical profile a, beta).

    Requires: channels identical, k[:,2] == -k[:,0], k[0,1] == k[2,1] == 0.
    Returns (a, beta) with a = k[:,0] (vertical mixing profile) and
    beta = k[1,1] + 1 (center coefficient incl. the residual).
    """
    k = np.asarray(kern, dtype=np.float32)
    if k.ndim != 4 or k.shape != (3, 3, 1, CH):
        return None
    if not np.all(k == k[:, :, :, :1]):
        return None
    k2 = k[:, :, 0, 0]
    if not (np.all(k2[:, 2] == -k2[:, 0]) and k2[0, 1] == 0 and k2[2, 1] == 0):
        return None
    return k2[:, 0].copy(), float(k2[1, 1]) + 1.0


def _numpy_fallback(x, kern):
    """Straightforward shifted-add implementation (safety net only)."""
    k = np.asarray(kern, dtype=np.float32)[:, :, 0, :]  # (3,3,CH)
    xp = np.pad(x, ((0, 0), (1, 1), (1, 1), (0, 0)))
    out = x.astype(np.float32).copy()
    for dh in range(3):
        for dw in range(3):
            out += k[dh, dw] * xp[:, dh : dh + H, dw : dw + W, :]
    return out


def _ensure_ntff_hook():
    """The agent image's antenv lacks axon_hooks; synthesize it so
    run_bass_kernel_spmd(trace=True) can reach the NTFF profiler."""
    import types

    if "antenv.axon_hooks" in sys.modules:
        return
    import antenv

    mod = types.ModuleType("antenv.axon_hooks")
    state = {}
    mod.set_axon_ntff_profile_hook = lambda h: state.__setitem__("h", h)
    mod.get_axon_ntff_profile_hook = lambda: state.get("h")
    sys.modules["antenv.axon_hooks"] = mod
    antenv.axon_hooks = mod
    try:
        if "/root/.axon_site" not in sys.path:
            sys.path.insert(0, "/root/.axon_site")
        from trn_agent_boot.trn_boot import _ntff_profile_via_ctypes

        hook = _ntff_profile_via_ctypes("/opt/axon/libaxon_pjrt.so")
        if hook is not None:
            mod.set_axon_ntff_profile_hook(hook)
    except Exception:
        pass


def _run_on_hw(x, a, beta, trace=False):
    global LAST_RESULTS
    if trace:
        _ensure_ntff_hook()
    import ml_dtypes
    from concourse.bass_utils import run_bass_kernel_spmd

    bf16 = ml_dtypes.bfloat16

    # vertical banded matrix: V[i, j] = coeff of x-row i in t-row j
    V = np.zeros((H, H), dtype=np.float32)
    idx = np.arange(H)
    V[idx[:-1] + 1, idx[:-1]] += a[2]   # i = j+1
    V[idx, idx] += a[1]                 # i = j
    V[idx[1:] - 1, idx[1:]] += a[0]     # i = j-1
    Vb = V.astype(bf16)

    key = (a.tobytes(), float(beta))
    if key not in _CACHE:
        _CACHE[key] = _build_bass(float(beta))
    nc = _CACHE[key]

    # host-side bf16 conversion + zero padding of one w column each side
    xp = np.zeros((N_CORES, ROWS, SLAB), dtype=bf16)
    xp[:, :, PAD : PAD + FS] = x.reshape(N_CORES, ROWS, FS).astype(bf16)
    in_maps = [{"x": xp[c], "vmat": Vb} for c in range(N_CORES)]
    res = run_bass_kernel_spmd(nc, in_maps, list(range(N_CORES)), trace=trace)
    LAST_RESULTS = res
    # device returned out/2 in bf16; x2 after upconvert is exact
    out = np.stack(
        [np.asarray(res.results[c]["out"], dtype=np.float32) for c in range(N_CORES)]
    )
    out *= 2.0
    return out.reshape(N, H, W, CH)


def kernel(x, kernel=None, _trace=False, **_unused):
    x = np.ascontiguousarray(np.asarray(x, dtype=np.float32))
    assert x.shape == (N, H, W, CH), f"unexpected x shape {x.shape}"
    if kernel is None:
        base = np.array(
            [[1.0, 0.0, -1.0], [0.0, 1.0, 0.0], [-1.0, 0.0, 1.0]], dtype=np.float32
        )
        kernel = np.tile(base[:, :, None, None], (1, 1, 1, CH))
    params = _stencil_params(kernel)
    if params is None:
        return _numpy_fallback(x, kernel)
    a, beta = params
    return _run_on_hw(x, a, beta, trace=_trace)


if __name__ == "__main__":
    xs = np.random.randn(N, H, W, CH).astype(np.float32)
    out = kernel(xs)
    print(out.shape, out.dtype)
